# revision 3
# baseline (speedup 1.0000x reference)
"""MoCo (topk_masking) Trainium2 kernel.

Strategy: tensor-shard the K=65536 queue column dimension across 8 NeuronCores
(8192 columns each). Per core:
  - l_neg/T slice  = (qn/T) @ queue[:, sl]        fp32 matmul (graded output)
  - backbone sims  = qb_n    @ queue_bb[:, sl]    bf16 matmul (only used to
    pick top-5 neighbors; bf16 input-rounding error ~4e-5 is far below the
    typical gap between the 5th/6th order statistics ~1e-3)
  - per 512-col chunk: top-8 values+indices (Max8/MaxIndex on VectorE)
  - exp-sum of l_neg/T per row (ACT Exp with accum_out) for the logsumexp
Host: l_pos, logits assembly, logsumexp/topk merge across cores, loss.

Self-contained: hardcodes shapes from the problem spec.
"""

import numpy as np
import ml_dtypes

import concourse.bass as bass
import concourse.mybir as mybir
import concourse.tile as tile
from concourse.bass import ds, ts
from concourse.bass_utils import run_bass_kernel_spmd

T = 0.07
TOPK = 5
B, NCROP, DIM, DIM_MLP, KQ = 256, 5, 128, 2048, 65536
N = B * NCROP  # 1280 rows
N_CORES = 8
KSL = KQ // N_CORES  # 8192 columns per core
CHUNK = 512
NCHUNK = KSL // CHUNK  # 16
RT = N // 128  # 10 row tiles
KS = DIM_MLP // 128  # 16 contraction slices for the backbone matmul

F32 = mybir.dt.float32
BF16 = mybir.dt.bfloat16
U32 = mybir.dt.uint32

_PATCHED = False


def _patch_tile_drain():
    """The walrus build here rejects >1 sync-wait per instruction; Tile's
    kernel-tail drain aggregates one wait per outstanding proc. Spread the
    extra waits over follow-up sequencer drains (executed in order on SP
    before the all-engine barrier — semantically identical)."""
    global _PATCHED
    if _PATCHED:
        return
    _PATCHED = True
    from concourse.vector_clock import ScopedClock

    def _drain_and_barrier(self, tick_clock, wait_clock):
        nc = self.nc
        drain_inst = nc.sync.drain()
        wait_clock.add_sem_waits(
            drain_inst.ins, ScopedClock({None: tick_clock.global_clock})
        )
        si = drain_inst.ins.sync_info
        if si is not None and len(si.on_wait) > 1:
            waits = list(si.on_wait)
            drain_inst.ins.sync_info = mybir.SyncInfo(
                on_wait=waits[:1], on_update=list(si.on_update)
            )
            for w in waits[1:]:
                extra = nc.sync.drain()
                extra.ins.sync_info = mybir.SyncInfo(on_wait=[w], on_update=[])
        nc.all_engine_barrier()
        assert self.sems is not None
        popped = nc._tile_sem_poison_stack.pop()
        assert popped is self._sem_poison
        nc.clear_and_free_semaphores(list(self.sems.allocated().values()))
        nc.all_engine_barrier()

    tile.TileContext._drain_and_barrier = _drain_and_barrier


def _split_multi_waits(nc):
    """The walrus build here accepts at most one sync-wait per instruction.
    Tile's scheduler can attach several. Move the extras onto same-engine
    InstNoOp wait-carriers inserted immediately before — the sequencer
    processes them in order, so semantics are unchanged."""
    for f in nc.m.functions:
        for b in f.blocks:
            new = []
            for inst in b.instructions:
                si = inst.sync_info
                if si is not None and len(si.on_wait) > 1:
                    waits = list(si.on_wait)
                    for j, w in enumerate(waits[:-1]):
                        new.append(
                            mybir.InstNoOp(
                                name=f"{inst.name}-w{j}",
                                engine=inst.engine,
                                bass_nofuse=True,
                                sync_info=mybir.SyncInfo(on_wait=[w], on_update=[]),
                            )
                        )
                    inst.sync_info = mybir.SyncInfo(
                        on_wait=waits[-1:], on_update=list(si.on_update)
                    )
                new.append(inst)
            b.instructions = new


def build_nc():
    _patch_tile_drain()
    nc = bass.Bass()
    qnt = nc.declare_dram_parameter("qnt", [DIM, N], F32, isOutput=False)
    qbt = nc.declare_dram_parameter("qbt", [DIM_MLP, N], BF16, isOutput=False)
    wq = nc.declare_dram_parameter("wq", [DIM, KSL], F32, isOutput=False)
    wb = nc.declare_dram_parameter("wb", [DIM_MLP, KSL], BF16, isOutput=False)
    logits_s = nc.declare_dram_parameter("logits_s", [N, KSL], F32, isOutput=True)
    cand_v = nc.declare_dram_parameter("cand_v", [N, NCHUNK * 8], F32, isOutput=True)
    cand_i = nc.declare_dram_parameter("cand_i", [N, NCHUNK * 8], U32, isOutput=True)
    esum = nc.declare_dram_parameter("esum", [N, 1], F32, isOutput=True)

    wb_r = wb.rearrange("(ks p) n -> p ks n", p=128)
    qbt_r = qbt.rearrange("(ks p) m -> p ks m", p=128)

    with tile.TileContext(nc) as tc:
        with (
            tc.tile_pool(name="resident", bufs=1) as res_pool,
            tc.tile_pool(name="wbchunk", bufs=2) as wb_pool,
            tc.tile_pool(name="lnout", bufs=3) as ln_pool,
            tc.tile_pool(name="expscr", bufs=2) as exp_pool,
            tc.tile_pool(name="lb_ps", bufs=3, space="PSUM") as lb_psum,
            tc.tile_pool(name="ln_ps", bufs=2, space="PSUM") as ln_psum,
        ):
            # resident inputs
            qnt_s = res_pool.tile([128, N], F32, tag="qnt")
            nc.sync.dma_start(qnt_s[:], qnt[:])
            qbt_s = res_pool.tile([128, KS, N], BF16, tag="qbt")
            nc.sync.dma_start(qbt_s[:], qbt_r[:])
            wq_s = res_pool.tile([128, KSL], F32, tag="wq")
            nc.sync.dma_start(wq_s[:], wq[:])
            # result strips (persist across the whole kernel)
            candv_s = res_pool.tile([128, RT, NCHUNK * 8], F32, tag="candv")
            candi_s = res_pool.tile([128, RT, NCHUNK * 8], U32, tag="candi")
            esum_strip = res_pool.tile([128, RT, NCHUNK], F32, tag="esumstrip")
            esum_red = res_pool.tile([128, RT, 1], F32, tag="esumred")

            for c in range(NCHUNK):
                wb_t = wb_pool.tile([128, KS, CHUNK], BF16, tag="wb")
                nc.sync.dma_start(wb_t[:], wb_r[:, :, ds(c * CHUNK, CHUNK)])
                for r in range(RT):
                    # backbone similarities chunk [128 rows, 512 cols]
                    ps_lb = lb_psum.tile([128, CHUNK], F32, tag="lb")
                    for k in range(KS):
                        nc.tensor.matmul(
                            ps_lb[:],
                            qbt_s[:, k, ts(r, 128)],
                            wb_t[:, k, :],
                            start=(k == 0),
                            stop=(k == KS - 1),
                        )
                    nc.vector.max(out=candv_s[:, r, ds(c * 8, 8)], in_=ps_lb[:])
                    nc.vector.max_index(
                        out=candi_s[:, r, ds(c * 8, 8)],
                        in_max=candv_s[:, r, ds(c * 8, 8)],
                        in_values=ps_lb[:],
                    )
                    # l_neg/T chunk (1/T folded into qnt on host)
                    ps_ln = ln_psum.tile([128, CHUNK], F32, tag="ln")
                    nc.tensor.matmul(
                        ps_ln[:],
                        qnt_s[:, ts(r, 128)],
                        wq_s[:, ds(c * CHUNK, CHUNK)],
                        start=True,
                        stop=True,
                    )
                    ln_t = ln_pool.tile([128, CHUNK], F32, tag="ln_sb")
                    nc.scalar.copy(ln_t[:], ps_ln[:])
                    exp_t = exp_pool.tile([128, CHUNK], F32, tag="exp_sb")
                    nc.scalar.activation(
                        exp_t[:],
                        ps_ln[:],
                        mybir.ActivationFunctionType.Exp,
                        accum_out=esum_strip[:, r, ds(c, 1)],
                    )
                    nc.sync.dma_start(
                        logits_s[ts(r, 128), ds(c * CHUNK, CHUNK)], ln_t[:]
                    )

            for r in range(RT):
                nc.vector.reduce_sum(
                    out=esum_red[:, r, :],
                    in_=esum_strip[:, r, :],
                    axis=mybir.AxisListType.X,
                )
            nc.sync.dma_start(
                esum.rearrange("(r p) o -> p r o", p=128), esum_red[:]
            )
            nc.sync.dma_start(
                cand_v.rearrange("(r p) j -> p r j", p=128), candv_s[:]
            )
            nc.sync.dma_start(
                cand_i.rearrange("(r p) j -> p r j", p=128), candi_s[:]
            )
    _split_multi_waits(nc)
    return nc


_NC_CACHE = None


def _get_nc():
    global _NC_CACHE
    if _NC_CACHE is None:
        _NC_CACHE = build_nc()
    return _NC_CACHE


def _l2norm(x, axis=-1, eps=1e-12):
    n = np.sqrt(np.sum(x * x, axis=axis, keepdims=True))
    return x / np.maximum(n, eps)


def kernel(q, k, q_backbone, queue, queue_backbone, _trace=False, _tmpdir=None):
    qn = _l2norm(q.astype(np.float32)).reshape(N, DIM)
    kn = _l2norm(k.astype(np.float32))
    qb = _l2norm(q_backbone.astype(np.float32)).reshape(N, DIM_MLP)

    qnt_host = np.ascontiguousarray((qn / T).T)  # [128, 1280] f32, 1/T folded
    qbt_host = np.ascontiguousarray(qb.T).astype(ml_dtypes.bfloat16)  # [2048, 1280]
    wb_bf = queue_backbone.astype(ml_dtypes.bfloat16)

    in_maps = []
    for c in range(N_CORES):
        sl = slice(c * KSL, (c + 1) * KSL)
        in_maps.append(
            {
                "qnt": qnt_host,
                "qbt": qbt_host,
                "wq": np.ascontiguousarray(queue[:, sl]),
                "wb": np.ascontiguousarray(wb_bf[:, sl]),
            }
        )

    nc = _get_nc()
    res = run_bass_kernel_spmd(
        nc, in_maps, list(range(N_CORES)), trace=_trace, tmpdir=_tmpdir
    )

    # ---- host-side assembly ----
    l_pos = np.sum(qn.reshape(B, NCROP, DIM) * kn[:, None, :], axis=-1).reshape(N)
    logits = np.empty((N, KQ + 1), dtype=np.float32)
    logits[:, 0] = l_pos / T
    esum_tot = np.zeros((N,), dtype=np.float64)
    all_vals = []
    all_cols = []
    chunk_base = (np.arange(NCHUNK * 8, dtype=np.int64) // 8) * CHUNK
    for c in range(N_CORES):
        r = res.results[c]
        logits[:, 1 + c * KSL : 1 + (c + 1) * KSL] = r["logits_s"]
        esum_tot += r["esum"][:, 0].astype(np.float64)
        all_vals.append(r["cand_v"])
        all_cols.append(c * KSL + chunk_base[None, :] + r["cand_i"].astype(np.int64))
    vals = np.concatenate(all_vals, axis=1)  # [N, 1024]
    cols = np.concatenate(all_cols, axis=1)
    top5 = np.argpartition(-vals, TOPK, axis=1)[:, :TOPK]
    sel_cols = np.take_along_axis(cols, top5, axis=1)  # [N, 5] global queue cols
    lse = np.log(esum_tot)  # [N] float64
    pos_logp = (
        logits[np.arange(N)[:, None], 1 + sel_cols].astype(np.float64) - lse[:, None]
    )
    loss = -np.mean(np.mean(pos_logp, axis=1))

    labels = np.zeros((N,), dtype=np.int32)
    out = (logits, labels, np.float32(loss))
    if _trace:
        return out, res
    return out


# revision 9
# speedup vs baseline: 1.8461x; 1.8461x over previous
"""MoCo (topk_masking) Trainium2 kernel.

Strategy: tensor-shard the K=65536 queue column dimension across 8 NeuronCores
(8192 columns each). Per core:
  - l_neg/T slice  = (qn/T) @ queue[:, sl]        fp32 matmul (graded output)
  - backbone sims  = qb_n    @ queue_bb[:, sl]    bf16 matmul (only used to
    pick top-5 neighbors; bf16 input-rounding error ~4e-5 is far below the
    typical gap between the 5th/6th order statistics ~1e-3)
  - per 512-col chunk: top-8 values+indices (Max8/MaxIndex on VectorE)
  - exp-sum of l_neg/T per row (ACT Exp with accum_out) for the logsumexp
Host: l_pos, logits assembly, logsumexp/topk merge across cores, loss.

Self-contained: hardcodes shapes from the problem spec.
"""

import numpy as np
import ml_dtypes

import concourse.bass as bass
import concourse.mybir as mybir
import concourse.tile as tile
from concourse.bass import ds, ts
from concourse.bass_utils import run_bass_kernel_spmd

T = 0.07
TOPK = 5
B, NCROP, DIM, DIM_MLP, KQ = 256, 5, 128, 2048, 65536
N = B * NCROP  # 1280 rows
N_CORES = 8
KSL = KQ // N_CORES  # 8192 columns per core
CHUNK = 512
NCHUNK = KSL // CHUNK  # 16
RT = N // 128  # 10 row tiles
KS = DIM_MLP // 128  # 16 contraction slices for the backbone matmul

F32 = mybir.dt.float32
F32R = mybir.dt.float32r  # same fp32 bits; PE multiplies at reduced precision
BF16 = mybir.dt.bfloat16
FP8 = mybir.dt.float8e4
U32 = mybir.dt.uint32

USE_FP8 = True  # fp8+DoubleRow backbone (selection only; host rescores top-16)
FP8_SCALE = 64.0  # lift ~N(0, 1/2048) entries into fp8e4m3's normal range
RESCORE = 16  # candidates per row to rescore exactly on host

_PATCHED = False


def _patch_tile_drain():
    """The walrus build here rejects >1 sync-wait per instruction; Tile's
    kernel-tail drain aggregates one wait per outstanding proc. Spread the
    extra waits over follow-up sequencer drains (executed in order on SP
    before the all-engine barrier — semantically identical)."""
    global _PATCHED
    if _PATCHED:
        return
    _PATCHED = True
    from concourse.vector_clock import ScopedClock

    def _drain_and_barrier(self, tick_clock, wait_clock):
        nc = self.nc
        drain_inst = nc.sync.drain()
        wait_clock.add_sem_waits(
            drain_inst.ins, ScopedClock({None: tick_clock.global_clock})
        )
        si = drain_inst.ins.sync_info
        if si is not None and len(si.on_wait) > 1:
            waits = list(si.on_wait)
            drain_inst.ins.sync_info = mybir.SyncInfo(
                on_wait=waits[:1], on_update=list(si.on_update)
            )
            for w in waits[1:]:
                extra = nc.sync.drain()
                extra.ins.sync_info = mybir.SyncInfo(on_wait=[w], on_update=[])
        nc.all_engine_barrier()
        assert self.sems is not None
        popped = nc._tile_sem_poison_stack.pop()
        assert popped is self._sem_poison
        nc.clear_and_free_semaphores(list(self.sems.allocated().values()))
        nc.all_engine_barrier()

    tile.TileContext._drain_and_barrier = _drain_and_barrier


def _split_multi_waits(nc):
    """The walrus build here accepts at most one sync-wait per instruction.
    Tile's scheduler can attach several. Move the extras onto same-engine
    InstNoOp wait-carriers inserted immediately before — the sequencer
    processes them in order, so semantics are unchanged."""
    for f in nc.m.functions:
        for b in f.blocks:
            new = []
            for inst in b.instructions:
                si = inst.sync_info
                if si is not None and len(si.on_wait) > 1:
                    waits = list(si.on_wait)
                    for j, w in enumerate(waits[:-1]):
                        new.append(
                            mybir.InstNoOp(
                                name=f"{inst.name}-w{j}",
                                engine=inst.engine,
                                bass_nofuse=True,
                                sync_info=mybir.SyncInfo(on_wait=[w], on_update=[]),
                            )
                        )
                    inst.sync_info = mybir.SyncInfo(
                        on_wait=waits[-1:], on_update=list(si.on_update)
                    )
                new.append(inst)
            b.instructions = new


def build_nc():
    _patch_tile_drain()
    nc = bass.Bass()
    bdt = FP8 if USE_FP8 else BF16
    qnt = nc.declare_dram_parameter("qnt", [DIM, N], F32R, isOutput=False)
    qbt = nc.declare_dram_parameter("qbt", [DIM_MLP, N], bdt, isOutput=False)
    wq = nc.declare_dram_parameter("wq", [DIM, KSL], F32R, isOutput=False)
    wb = nc.declare_dram_parameter("wb", [DIM_MLP, KSL], bdt, isOutput=False)
    logits_s = nc.declare_dram_parameter("logits_s", [N, KSL], F32, isOutput=True)
    cand_v = nc.declare_dram_parameter("cand_v", [N, NCHUNK * 8], F32, isOutput=True)
    cand_i = nc.declare_dram_parameter("cand_i", [N, NCHUNK * 8], U32, isOutput=True)
    esum = nc.declare_dram_parameter("esum", [N, 1], F32, isOutput=True)

    wb_r = wb.rearrange("(ks p) n -> p ks n", p=128)
    qbt_r = qbt.rearrange("(ks p) m -> p ks m", p=128)

    with tile.TileContext(nc) as tc:
        with (
            tc.tile_pool(name="resident", bufs=1) as res_pool,
            tc.tile_pool(name="wbchunk", bufs=2) as wb_pool,
            tc.tile_pool(name="lnout", bufs=3) as ln_pool,
            tc.tile_pool(name="expscr", bufs=2) as exp_pool,
            tc.tile_pool(name="lb_ps", bufs=3, space="PSUM") as lb_psum,
            tc.tile_pool(name="ln_ps", bufs=2, space="PSUM") as ln_psum,
        ):
            # resident inputs
            qnt_s = res_pool.tile([128, N], F32R, tag="qnt")
            nc.sync.dma_start(qnt_s[:], qnt[:])
            qbt_s = res_pool.tile([128, KS, N], bdt, tag="qbt")
            nc.sync.dma_start(qbt_s[:], qbt_r[:])
            wq_s = res_pool.tile([128, KSL], F32R, tag="wq")
            nc.sync.dma_start(wq_s[:], wq[:])
            # result strips (persist across the whole kernel)
            candv_s = res_pool.tile([128, RT, NCHUNK * 8], F32, tag="candv")
            candi_s = res_pool.tile([128, RT, NCHUNK * 8], U32, tag="candi")
            esum_strip = res_pool.tile([128, RT, NCHUNK], F32, tag="esumstrip")
            esum_red = res_pool.tile([128, RT, 1], F32, tag="esumred")

            kstep = 2 if USE_FP8 else 1
            pmode = mybir.MatmulPerfMode.DoubleRow if USE_FP8 else None
            for c in range(NCHUNK):
                wb_t = wb_pool.tile([128, KS, CHUNK], bdt, tag="wb")
                nc.sync.dma_start(wb_t[:], wb_r[:, :, ds(c * CHUNK, CHUNK)])
                for r in range(RT):
                    # backbone similarities chunk [128 rows, 512 cols]
                    ps_lb = lb_psum.tile([128, CHUNK], F32, tag="lb")
                    for k in range(0, KS, kstep):
                        lhsT = (
                            qbt_s[:, k : k + 2, ts(r, 128)]
                            if USE_FP8
                            else qbt_s[:, k, ts(r, 128)]
                        )
                        rhs = wb_t[:, k : k + 2, :] if USE_FP8 else wb_t[:, k, :]
                        nc.tensor.matmul(
                            ps_lb[:],
                            lhsT,
                            rhs,
                            start=(k == 0),
                            stop=(k + kstep >= KS),
                            perf_mode=pmode,
                        )
                    nc.vector.max(out=candv_s[:, r, ds(c * 8, 8)], in_=ps_lb[:])
                    nc.vector.max_index(
                        out=candi_s[:, r, ds(c * 8, 8)],
                        in_max=candv_s[:, r, ds(c * 8, 8)],
                        in_values=ps_lb[:],
                    )
                    # l_neg/T chunk (1/T folded into qnt on host)
                    ps_ln = ln_psum.tile([128, CHUNK], F32, tag="ln")
                    nc.tensor.matmul(
                        ps_ln[:],
                        qnt_s[:, ts(r, 128)],
                        wq_s[:, ds(c * CHUNK, CHUNK)],
                        start=True,
                        stop=True,
                    )
                    ln_t = ln_pool.tile([128, CHUNK], F32, tag="ln_sb")
                    nc.scalar.copy(ln_t[:], ps_ln[:])
                    exp_t = exp_pool.tile([128, CHUNK], F32, tag="exp_sb")
                    nc.scalar.activation(
                        exp_t[:],
                        ps_ln[:],
                        mybir.ActivationFunctionType.Exp,
                        accum_out=esum_strip[:, r, ds(c, 1)],
                    )
                    nc.sync.dma_start(
                        logits_s[ts(r, 128), ds(c * CHUNK, CHUNK)], ln_t[:]
                    )

            for r in range(RT):
                nc.vector.reduce_sum(
                    out=esum_red[:, r, :],
                    in_=esum_strip[:, r, :],
                    axis=mybir.AxisListType.X,
                )
            nc.sync.dma_start(
                esum.rearrange("(r p) o -> p r o", p=128), esum_red[:]
            )
            nc.sync.dma_start(
                cand_v.rearrange("(r p) j -> p r j", p=128), candv_s[:]
            )
            nc.sync.dma_start(
                cand_i.rearrange("(r p) j -> p r j", p=128), candi_s[:]
            )
    _split_multi_waits(nc)
    return nc


_NC_CACHE = None


def _get_nc():
    global _NC_CACHE
    if _NC_CACHE is None:
        _NC_CACHE = build_nc()
    return _NC_CACHE


def _l2norm(x, axis=-1, eps=1e-12):
    n = np.sqrt(np.sum(x * x, axis=axis, keepdims=True))
    return x / np.maximum(n, eps)


def kernel(q, k, q_backbone, queue, queue_backbone, _trace=False, _tmpdir=None):
    qn = _l2norm(q.astype(np.float32)).reshape(N, DIM)
    kn = _l2norm(k.astype(np.float32))
    qb = _l2norm(q_backbone.astype(np.float32)).reshape(N, DIM_MLP)

    qnt_host = np.ascontiguousarray((qn / T).T)  # [128, 1280] f32, 1/T folded
    bnp = ml_dtypes.float8_e4m3 if USE_FP8 else ml_dtypes.bfloat16
    bscale = FP8_SCALE if USE_FP8 else 1.0
    qbt_host = np.ascontiguousarray((qb * bscale).T).astype(bnp)  # [2048, 1280]
    wb_bf = (queue_backbone * np.float32(bscale)).astype(bnp)

    in_maps = []
    for c in range(N_CORES):
        sl = slice(c * KSL, (c + 1) * KSL)
        in_maps.append(
            {
                "qnt": qnt_host,
                "qbt": qbt_host,
                "wq": np.ascontiguousarray(queue[:, sl]),
                "wb": np.ascontiguousarray(wb_bf[:, sl]),
            }
        )

    nc = _get_nc()
    res = run_bass_kernel_spmd(
        nc, in_maps, list(range(N_CORES)), trace=_trace, tmpdir=_tmpdir
    )

    # ---- host-side assembly ----
    l_pos = np.sum(qn.reshape(B, NCROP, DIM) * kn[:, None, :], axis=-1).reshape(N)
    logits = np.empty((N, KQ + 1), dtype=np.float32)
    logits[:, 0] = l_pos / T
    esum_tot = np.zeros((N,), dtype=np.float64)
    all_vals = []
    all_cols = []
    chunk_base = (np.arange(NCHUNK * 8, dtype=np.int64) // 8) * CHUNK
    for c in range(N_CORES):
        r = res.results[c]
        logits[:, 1 + c * KSL : 1 + (c + 1) * KSL] = r["logits_s"]
        esum_tot += r["esum"][:, 0].astype(np.float64)
        all_vals.append(r["cand_v"])
        all_cols.append(c * KSL + chunk_base[None, :] + r["cand_i"].astype(np.int64))
    vals = np.concatenate(all_vals, axis=1)  # [N, 1024]
    cols = np.concatenate(all_cols, axis=1)
    if USE_FP8:
        # fp8 values only rank candidates; rescore the top-RESCORE exactly.
        topm = np.argpartition(-vals, RESCORE, axis=1)[:, :RESCORE]
        cand_cols = np.take_along_axis(cols, topm, axis=1)  # [N, 16]
        gathered = queue_backbone[:, cand_cols.ravel()]  # [2048, N*16]
        exact = np.einsum(
            "nd,dnm->nm",
            qb,
            gathered.reshape(DIM_MLP, N, RESCORE),
            optimize=True,
        )
        top5 = np.argpartition(-exact, TOPK, axis=1)[:, :TOPK]
        sel_cols = np.take_along_axis(cand_cols, top5, axis=1)
    else:
        top5 = np.argpartition(-vals, TOPK, axis=1)[:, :TOPK]
        sel_cols = np.take_along_axis(cols, top5, axis=1)  # [N, 5] global cols
    lse = np.log(esum_tot)  # [N] float64
    pos_logp = (
        logits[np.arange(N)[:, None], 1 + sel_cols].astype(np.float64) - lse[:, None]
    )
    loss = -np.mean(np.mean(pos_logp, axis=1))

    labels = np.zeros((N,), dtype=np.int32)
    out = (logits, labels, np.float32(loss))
    if _trace:
        return out, res
    return out


# revision 11
# speedup vs baseline: 1.8984x; 1.0283x over previous
"""MoCo (topk_masking) Trainium2 kernel.

Strategy: tensor-shard the K=65536 queue column dimension across 8 NeuronCores
(8192 columns each). Per core:
  - l_neg/T slice  = (qn/T) @ queue[:, sl]        fp32 matmul (graded output)
  - backbone sims  = qb_n    @ queue_bb[:, sl]    bf16 matmul (only used to
    pick top-5 neighbors; bf16 input-rounding error ~4e-5 is far below the
    typical gap between the 5th/6th order statistics ~1e-3)
  - per 512-col chunk: top-8 values+indices (Max8/MaxIndex on VectorE)
  - exp-sum of l_neg/T per row (ACT Exp with accum_out) for the logsumexp
Host: l_pos, logits assembly, logsumexp/topk merge across cores, loss.

Self-contained: hardcodes shapes from the problem spec.
"""

import numpy as np
import ml_dtypes

import concourse.bass as bass
import concourse.mybir as mybir
import concourse.tile as tile
from concourse.bass import ds, ts
from concourse.bass_utils import run_bass_kernel_spmd

T = 0.07
TOPK = 5
B, NCROP, DIM, DIM_MLP, KQ = 256, 5, 128, 2048, 65536
N = B * NCROP  # 1280 rows
N_CORES = 8
KSL = KQ // N_CORES  # 8192 columns per core
CHUNK = 512
NCHUNK = KSL // CHUNK  # 16
RT = N // 128  # 10 row tiles
KS = DIM_MLP // 128  # 16 contraction slices for the backbone matmul

F32 = mybir.dt.float32
F32R = mybir.dt.float32r  # same fp32 bits; PE multiplies at reduced precision
BF16 = mybir.dt.bfloat16
FP8 = mybir.dt.float8e4
U32 = mybir.dt.uint32

USE_FP8 = True  # fp8+DoubleRow backbone (selection only; host rescores top-16)
FP8_SCALE = 64.0  # lift ~N(0, 1/2048) entries into fp8e4m3's normal range
RESCORE = 16  # candidates per row to rescore exactly on host

_PATCHED = False


def _patch_tile_drain():
    """The walrus build here rejects >1 sync-wait per instruction; Tile's
    kernel-tail drain aggregates one wait per outstanding proc. Spread the
    extra waits over follow-up sequencer drains (executed in order on SP
    before the all-engine barrier — semantically identical)."""
    global _PATCHED
    if _PATCHED:
        return
    _PATCHED = True
    from concourse.vector_clock import ScopedClock

    def _drain_and_barrier(self, tick_clock, wait_clock):
        nc = self.nc
        drain_inst = nc.sync.drain()
        wait_clock.add_sem_waits(
            drain_inst.ins, ScopedClock({None: tick_clock.global_clock})
        )
        si = drain_inst.ins.sync_info
        if si is not None and len(si.on_wait) > 1:
            waits = list(si.on_wait)
            drain_inst.ins.sync_info = mybir.SyncInfo(
                on_wait=waits[:1], on_update=list(si.on_update)
            )
            for w in waits[1:]:
                extra = nc.sync.drain()
                extra.ins.sync_info = mybir.SyncInfo(on_wait=[w], on_update=[])
        nc.all_engine_barrier()
        assert self.sems is not None
        popped = nc._tile_sem_poison_stack.pop()
        assert popped is self._sem_poison
        nc.clear_and_free_semaphores(list(self.sems.allocated().values()))
        nc.all_engine_barrier()

    tile.TileContext._drain_and_barrier = _drain_and_barrier


def _split_multi_waits(nc):
    """The walrus build here accepts at most one sync-wait per instruction.
    Tile's scheduler can attach several. Move the extras onto same-engine
    InstNoOp wait-carriers inserted immediately before — the sequencer
    processes them in order, so semantics are unchanged."""
    for f in nc.m.functions:
        for b in f.blocks:
            new = []
            for inst in b.instructions:
                si = inst.sync_info
                if si is not None and len(si.on_wait) > 1:
                    waits = list(si.on_wait)
                    for j, w in enumerate(waits[:-1]):
                        new.append(
                            mybir.InstNoOp(
                                name=f"{inst.name}-w{j}",
                                engine=inst.engine,
                                bass_nofuse=True,
                                sync_info=mybir.SyncInfo(on_wait=[w], on_update=[]),
                            )
                        )
                    inst.sync_info = mybir.SyncInfo(
                        on_wait=waits[-1:], on_update=list(si.on_update)
                    )
                new.append(inst)
            b.instructions = new


def build_nc():
    _patch_tile_drain()
    nc = bass.Bass()
    bdt = FP8 if USE_FP8 else BF16
    qnt = nc.declare_dram_parameter("qnt", [DIM, N], F32R, isOutput=False)
    qbt = nc.declare_dram_parameter("qbt", [DIM_MLP, N], bdt, isOutput=False)
    wq = nc.declare_dram_parameter("wq", [DIM, KSL], F32R, isOutput=False)
    wb = nc.declare_dram_parameter("wb", [DIM_MLP, KSL], bdt, isOutput=False)
    logits_s = nc.declare_dram_parameter("logits_s", [N, KSL], F32, isOutput=True)
    cand_v = nc.declare_dram_parameter("cand_v", [N, NCHUNK * 8], F32, isOutput=True)
    cand_i = nc.declare_dram_parameter("cand_i", [N, NCHUNK * 8], U32, isOutput=True)
    esum = nc.declare_dram_parameter("esum", [N, 1], F32, isOutput=True)

    wb_r = wb.rearrange("(ks p) n -> p ks n", p=128)
    qbt_r = qbt.rearrange("(ks p) m -> p ks m", p=128)

    with tile.TileContext(nc) as tc:
        with (
            tc.tile_pool(name="resident", bufs=1) as res_pool,
            tc.tile_pool(name="wbchunk", bufs=2) as wb_pool,
            tc.tile_pool(name="lnout", bufs=2) as ln_pool,
            tc.tile_pool(name="expscr", bufs=2) as exp_pool,
            tc.tile_pool(name="lb_ps", bufs=3, space="PSUM") as lb_psum,
            tc.tile_pool(name="ln_ps", bufs=1, space="PSUM") as ln_psum,
        ):
            # resident inputs; split DMAs so the first matmuls start early
            qnt_s = res_pool.tile([128, N], F32R, tag="qnt")
            nc.sync.dma_start(qnt_s[:], qnt[:])
            qbt_s = res_pool.tile([128, KS, N], bdt, tag="qbt")
            wb0 = wb_pool.tile([128, KS, CHUNK], bdt, tag="wb")
            for k in range(KS):
                nc.sync.dma_start(wb0[:, k, :], wb_r[:, k, ds(0, CHUNK)])
                nc.sync.dma_start(qbt_s[:, k, :], qbt_r[:, k, :])
            wq_s = res_pool.tile([128, KSL], F32R, tag="wq")
            for c in range(NCHUNK):
                nc.sync.dma_start(
                    wq_s[:, ds(c * CHUNK, CHUNK)], wq[:, ds(c * CHUNK, CHUNK)]
                )
            # result strips (persist across the whole kernel)
            candv_s = res_pool.tile([128, RT, NCHUNK * 8], F32, tag="candv")
            candi_s = res_pool.tile([128, RT, NCHUNK * 8], U32, tag="candi")
            NQUAD = NCHUNK // 4  # l_neg processed in 2048-wide quad groups
            QW = 4 * CHUNK
            esum_strip = res_pool.tile([128, RT, NQUAD], F32, tag="esumstrip")
            esum_red = res_pool.tile([128, RT, 1], F32, tag="esumred")

            kstep = 2 if USE_FP8 else 1
            pmode = mybir.MatmulPerfMode.DoubleRow if USE_FP8 else None
            for c in range(NCHUNK):
                if c == 0:
                    wb_t = wb0
                else:
                    wb_t = wb_pool.tile([128, KS, CHUNK], bdt, tag="wb")
                    nc.sync.dma_start(wb_t[:], wb_r[:, :, ds(c * CHUNK, CHUNK)])
                for r in range(RT):
                    # backbone similarities chunk [128 rows, 512 cols]
                    ps_lb = lb_psum.tile([128, CHUNK], F32, tag="lb")
                    for k in range(0, KS, kstep):
                        lhsT = (
                            qbt_s[:, k : k + 2, ts(r, 128)]
                            if USE_FP8
                            else qbt_s[:, k, ts(r, 128)]
                        )
                        rhs = wb_t[:, k : k + 2, :] if USE_FP8 else wb_t[:, k, :]
                        nc.tensor.matmul(
                            ps_lb[:],
                            lhsT,
                            rhs,
                            start=(k == 0),
                            stop=(k + kstep >= KS),
                            perf_mode=pmode,
                        )
                    nc.vector.max(out=candv_s[:, r, ds(c * 8, 8)], in_=ps_lb[:])
                    nc.vector.max_index(
                        out=candi_s[:, r, ds(c * 8, 8)],
                        in_max=candv_s[:, r, ds(c * 8, 8)],
                        in_values=ps_lb[:],
                    )
                if c % 4 == 3:
                    # l_neg/T quad [128, 2048] (1/T folded into qnt on host)
                    cq = c // 4
                    for r in range(RT):
                        ps_ln = ln_psum.tile([128, QW], F32, tag="ln")
                        for j in range(4):
                            nc.tensor.matmul(
                                ps_ln[:, ds(j * CHUNK, CHUNK)],
                                qnt_s[:, ts(r, 128)],
                                wq_s[:, ds((cq * 4 + j) * CHUNK, CHUNK)],
                                start=True,
                                stop=True,
                            )
                        ln_t = ln_pool.tile([128, QW], F32, tag="ln_sb")
                        nc.scalar.copy(ln_t[:], ps_ln[:])
                        exp_t = exp_pool.tile([128, QW], F32, tag="exp_sb")
                        nc.scalar.activation(
                            exp_t[:],
                            ps_ln[:],
                            mybir.ActivationFunctionType.Exp,
                            accum_out=esum_strip[:, r, ds(cq, 1)],
                        )
                        nc.sync.dma_start(
                            logits_s[ts(r, 128), ds(cq * QW, QW)], ln_t[:]
                        )

            for r in range(RT):
                nc.vector.reduce_sum(
                    out=esum_red[:, r, :],
                    in_=esum_strip[:, r, :],
                    axis=mybir.AxisListType.X,
                )
            nc.sync.dma_start(
                esum.rearrange("(r p) o -> p r o", p=128), esum_red[:]
            )
            nc.sync.dma_start(
                cand_v.rearrange("(r p) j -> p r j", p=128), candv_s[:]
            )
            nc.sync.dma_start(
                cand_i.rearrange("(r p) j -> p r j", p=128), candi_s[:]
            )
    _split_multi_waits(nc)
    return nc


_NC_CACHE = None


def _get_nc():
    global _NC_CACHE
    if _NC_CACHE is None:
        _NC_CACHE = build_nc()
    return _NC_CACHE


def _l2norm(x, axis=-1, eps=1e-12):
    n = np.sqrt(np.sum(x * x, axis=axis, keepdims=True))
    return x / np.maximum(n, eps)


def kernel(q, k, q_backbone, queue, queue_backbone, _trace=False, _tmpdir=None):
    qn = _l2norm(q.astype(np.float32)).reshape(N, DIM)
    kn = _l2norm(k.astype(np.float32))
    qb = _l2norm(q_backbone.astype(np.float32)).reshape(N, DIM_MLP)

    qnt_host = np.ascontiguousarray((qn / T).T)  # [128, 1280] f32, 1/T folded
    bnp = ml_dtypes.float8_e4m3 if USE_FP8 else ml_dtypes.bfloat16
    bscale = FP8_SCALE if USE_FP8 else 1.0
    qbt_host = np.ascontiguousarray((qb * bscale).T).astype(bnp)  # [2048, 1280]
    wb_bf = (queue_backbone * np.float32(bscale)).astype(bnp)

    in_maps = []
    for c in range(N_CORES):
        sl = slice(c * KSL, (c + 1) * KSL)
        in_maps.append(
            {
                "qnt": qnt_host,
                "qbt": qbt_host,
                "wq": np.ascontiguousarray(queue[:, sl]),
                "wb": np.ascontiguousarray(wb_bf[:, sl]),
            }
        )

    nc = _get_nc()
    res = run_bass_kernel_spmd(
        nc, in_maps, list(range(N_CORES)), trace=_trace, tmpdir=_tmpdir
    )

    # ---- host-side assembly ----
    l_pos = np.sum(qn.reshape(B, NCROP, DIM) * kn[:, None, :], axis=-1).reshape(N)
    logits = np.empty((N, KQ + 1), dtype=np.float32)
    logits[:, 0] = l_pos / T
    esum_tot = np.zeros((N,), dtype=np.float64)
    all_vals = []
    all_cols = []
    chunk_base = (np.arange(NCHUNK * 8, dtype=np.int64) // 8) * CHUNK
    for c in range(N_CORES):
        r = res.results[c]
        logits[:, 1 + c * KSL : 1 + (c + 1) * KSL] = r["logits_s"]
        esum_tot += r["esum"][:, 0].astype(np.float64)
        all_vals.append(r["cand_v"])
        all_cols.append(c * KSL + chunk_base[None, :] + r["cand_i"].astype(np.int64))
    vals = np.concatenate(all_vals, axis=1)  # [N, 1024]
    cols = np.concatenate(all_cols, axis=1)
    if USE_FP8:
        # fp8 values only rank candidates; rescore the top-RESCORE exactly.
        topm = np.argpartition(-vals, RESCORE, axis=1)[:, :RESCORE]
        cand_cols = np.take_along_axis(cols, topm, axis=1)  # [N, 16]
        gathered = queue_backbone[:, cand_cols.ravel()]  # [2048, N*16]
        exact = np.einsum(
            "nd,dnm->nm",
            qb,
            gathered.reshape(DIM_MLP, N, RESCORE),
            optimize=True,
        )
        top5 = np.argpartition(-exact, TOPK, axis=1)[:, :TOPK]
        sel_cols = np.take_along_axis(cand_cols, top5, axis=1)
    else:
        top5 = np.argpartition(-vals, TOPK, axis=1)[:, :TOPK]
        sel_cols = np.take_along_axis(cols, top5, axis=1)  # [N, 5] global cols
    lse = np.log(esum_tot)  # [N] float64
    pos_logp = (
        logits[np.arange(N)[:, None], 1 + sel_cols].astype(np.float64) - lse[:, None]
    )
    loss = -np.mean(np.mean(pos_logp, axis=1))

    labels = np.zeros((N,), dtype=np.int32)
    out = (logits, labels, np.float32(loss))
    if _trace:
        return out, res
    return out


# revision 14
# speedup vs baseline: 1.9538x; 1.0292x over previous
"""MoCo (topk_masking) Trainium2 kernel.

Strategy: tensor-shard the K=65536 queue column dimension across 8 NeuronCores
(8192 columns each). Per core:
  - l_neg/T slice  = (qn/T) @ queue[:, sl]        fp32 matmul (graded output)
  - backbone sims  = qb_n    @ queue_bb[:, sl]    bf16 matmul (only used to
    pick top-5 neighbors; bf16 input-rounding error ~4e-5 is far below the
    typical gap between the 5th/6th order statistics ~1e-3)
  - per 512-col chunk: top-8 values+indices (Max8/MaxIndex on VectorE)
  - exp-sum of l_neg/T per row (ACT Exp with accum_out) for the logsumexp
Host: l_pos, logits assembly, logsumexp/topk merge across cores, loss.

Self-contained: hardcodes shapes from the problem spec.
"""

import numpy as np
import ml_dtypes

import concourse.bass as bass
import concourse.mybir as mybir
import concourse.tile as tile
from concourse.bass import ds, ts
from concourse.bass_utils import run_bass_kernel_spmd

T = 0.07
TOPK = 5
B, NCROP, DIM, DIM_MLP, KQ = 256, 5, 128, 2048, 65536
N = B * NCROP  # 1280 rows
N_CORES = 8
KSL = KQ // N_CORES  # 8192 columns per core
CHUNK = 512
NCHUNK = KSL // CHUNK  # 16
RT = N // 128  # 10 row tiles
KS = DIM_MLP // 128  # 16 contraction slices for the backbone matmul

F32 = mybir.dt.float32
F32R = mybir.dt.float32r  # same fp32 bits; PE multiplies at reduced precision
BF16 = mybir.dt.bfloat16
FP8 = mybir.dt.float8e4
U32 = mybir.dt.uint32

USE_FP8 = True  # fp8+DoubleRow backbone (selection only; host rescores top-16)
FP8_SCALE = 64.0  # lift ~N(0, 1/2048) entries into fp8e4m3's normal range
RESCORE = 16  # candidates per row to rescore exactly on host

_PATCHED = False


def _patch_tile_drain():
    """The walrus build here rejects >1 sync-wait per instruction; Tile's
    kernel-tail drain aggregates one wait per outstanding proc. Spread the
    extra waits over follow-up sequencer drains (executed in order on SP
    before the all-engine barrier — semantically identical)."""
    global _PATCHED
    if _PATCHED:
        return
    _PATCHED = True
    from concourse.vector_clock import ScopedClock

    def _drain_and_barrier(self, tick_clock, wait_clock):
        nc = self.nc
        drain_inst = nc.sync.drain()
        wait_clock.add_sem_waits(
            drain_inst.ins, ScopedClock({None: tick_clock.global_clock})
        )
        si = drain_inst.ins.sync_info
        if si is not None and len(si.on_wait) > 1:
            waits = list(si.on_wait)
            drain_inst.ins.sync_info = mybir.SyncInfo(
                on_wait=waits[:1], on_update=list(si.on_update)
            )
            for w in waits[1:]:
                extra = nc.sync.drain()
                extra.ins.sync_info = mybir.SyncInfo(on_wait=[w], on_update=[])
        nc.all_engine_barrier()
        assert self.sems is not None
        popped = nc._tile_sem_poison_stack.pop()
        assert popped is self._sem_poison
        nc.clear_and_free_semaphores(list(self.sems.allocated().values()))
        nc.all_engine_barrier()

    tile.TileContext._drain_and_barrier = _drain_and_barrier


def _split_multi_waits(nc):
    """The walrus build here accepts at most one sync-wait per instruction.
    Tile's scheduler can attach several. Move the extras onto same-engine
    InstNoOp wait-carriers inserted immediately before — the sequencer
    processes them in order, so semantics are unchanged."""
    for f in nc.m.functions:
        for b in f.blocks:
            new = []
            for inst in b.instructions:
                si = inst.sync_info
                if si is not None and len(si.on_wait) > 1:
                    waits = list(si.on_wait)
                    for j, w in enumerate(waits[:-1]):
                        new.append(
                            mybir.InstNoOp(
                                name=f"{inst.name}-w{j}",
                                engine=inst.engine,
                                bass_nofuse=True,
                                sync_info=mybir.SyncInfo(on_wait=[w], on_update=[]),
                            )
                        )
                    inst.sync_info = mybir.SyncInfo(
                        on_wait=waits[-1:], on_update=list(si.on_update)
                    )
                new.append(inst)
            b.instructions = new


def build_nc():
    _patch_tile_drain()
    nc = bass.Bass()
    bdt = FP8 if USE_FP8 else BF16
    qnt = nc.declare_dram_parameter("qnt", [DIM, N], F32R, isOutput=False)
    qbt = nc.declare_dram_parameter("qbt", [DIM_MLP, N], bdt, isOutput=False)
    wq = nc.declare_dram_parameter("wq", [DIM, KSL], F32R, isOutput=False)
    wb = nc.declare_dram_parameter("wb", [DIM_MLP, KSL], bdt, isOutput=False)
    logits_s = nc.declare_dram_parameter("logits_s", [N, KSL], F32, isOutput=True)
    cand_v = nc.declare_dram_parameter("cand_v", [N, NCHUNK * 8], F32, isOutput=True)
    cand_i = nc.declare_dram_parameter("cand_i", [N, NCHUNK * 8], U32, isOutput=True)
    esum = nc.declare_dram_parameter("esum", [N, 1], F32, isOutput=True)

    wb_r = wb.rearrange("(ks p) n -> p ks n", p=128)
    qbt_r = qbt.rearrange("(ks p) m -> p ks m", p=128)

    with tile.TileContext(nc) as tc:
        with (
            tc.tile_pool(name="resident", bufs=1) as res_pool,
            tc.tile_pool(name="wbchunk", bufs=2) as wb_pool,
            tc.tile_pool(name="lnout", bufs=2) as ln_pool,
            tc.tile_pool(name="expscr", bufs=2) as exp_pool,
            tc.tile_pool(name="lb_ps", bufs=4, space="PSUM") as lb_psum,
            tc.tile_pool(name="ln_ps", bufs=1, space="PSUM") as ln_psum,
        ):
            # resident inputs; split DMAs so the first matmuls start early
            qnt_s = res_pool.tile([128, N], F32R, tag="qnt")
            nc.sync.dma_start(qnt_s[:], qnt[:])
            qbt_s = res_pool.tile([128, KS, N], bdt, tag="qbt")
            wb0 = wb_pool.tile([128, KS, CHUNK], bdt, tag="wb")
            for k in range(KS):
                nc.sync.dma_start(wb0[:, k, :], wb_r[:, k, ds(0, CHUNK)])
                nc.sync.dma_start(qbt_s[:, k, :], qbt_r[:, k, :])
            wq_s = res_pool.tile([128, KSL], F32R, tag="wq")
            for c in range(NCHUNK):
                nc.sync.dma_start(
                    wq_s[:, ds(c * CHUNK, CHUNK)], wq[:, ds(c * CHUNK, CHUNK)]
                )
            # result strips (persist across the whole kernel)
            candv_s = res_pool.tile([128, RT, NCHUNK * 8], F32, tag="candv")
            candi_s = res_pool.tile([128, RT, NCHUNK * 8], U32, tag="candi")
            NQUAD = NCHUNK // 4  # l_neg processed in 2048-wide quad groups
            QW = 4 * CHUNK
            esum_strip = res_pool.tile([128, RT, NQUAD], F32, tag="esumstrip")
            esum_red = res_pool.tile([128, RT, 1], F32, tag="esumred")

            kstep = 2 if USE_FP8 else 1
            pmode = mybir.MatmulPerfMode.DoubleRow if USE_FP8 else None
            for c in range(NCHUNK):
                if c == 0:
                    wb_t = wb0
                else:
                    wb_t = wb_pool.tile([128, KS, CHUNK], bdt, tag="wb")
                    nc.sync.dma_start(wb_t[:], wb_r[:, :, ds(c * CHUNK, CHUNK)])
                for r in range(RT):
                    # backbone similarities chunk [128 rows, 512 cols]
                    ps_lb = lb_psum.tile([128, CHUNK], F32, tag="lb")
                    for k in range(0, KS, kstep):
                        lhsT = (
                            qbt_s[:, k : k + 2, ts(r, 128)]
                            if USE_FP8
                            else qbt_s[:, k, ts(r, 128)]
                        )
                        rhs = wb_t[:, k : k + 2, :] if USE_FP8 else wb_t[:, k, :]
                        nc.tensor.matmul(
                            ps_lb[:],
                            lhsT,
                            rhs,
                            start=(k == 0),
                            stop=(k + kstep >= KS),
                            perf_mode=pmode,
                        )
                    nc.vector.max(out=candv_s[:, r, ds(c * 8, 8)], in_=ps_lb[:])
                    nc.vector.max_index(
                        out=candi_s[:, r, ds(c * 8, 8)],
                        in_max=candv_s[:, r, ds(c * 8, 8)],
                        in_values=ps_lb[:],
                    )
                    if c == NCHUNK - 1:
                        # stream result strips out as each row-tile finishes
                        nc.sync.dma_start(
                            cand_v.rearrange("(r p) j -> p r j", p=128)[:, r, :],
                            candv_s[:, r, :],
                        )
                        nc.sync.dma_start(
                            cand_i.rearrange("(r p) j -> p r j", p=128)[:, r, :],
                            candi_s[:, r, :],
                        )
                if c % 4 == 1:
                    # l_neg/T quad [128, 2048] (1/T folded into qnt on host)
                    cq = c // 4
                    for r in range(RT):
                        ps_ln = ln_psum.tile([128, QW], F32, tag="ln")
                        for j in range(4):
                            nc.tensor.matmul(
                                ps_ln[:, ds(j * CHUNK, CHUNK)],
                                qnt_s[:, ts(r, 128)],
                                wq_s[:, ds((cq * 4 + j) * CHUNK, CHUNK)],
                                start=True,
                                stop=True,
                            )
                        ln_t = ln_pool.tile([128, QW], F32, tag="ln_sb")
                        nc.scalar.copy(ln_t[:], ps_ln[:])
                        exp_t = exp_pool.tile([128, QW], F32, tag="exp_sb")
                        nc.scalar.activation(
                            exp_t[:],
                            ps_ln[:],
                            mybir.ActivationFunctionType.Exp,
                            accum_out=esum_strip[:, r, ds(cq, 1)],
                        )
                        nc.sync.dma_start(
                            logits_s[ts(r, 128), ds(cq * QW, QW)], ln_t[:]
                        )
                        if cq == NQUAD - 1:
                            nc.vector.reduce_sum(
                                out=esum_red[:, r, :],
                                in_=esum_strip[:, r, :],
                                axis=mybir.AxisListType.X,
                            )
                            nc.sync.dma_start(
                                esum.rearrange("(r p) o -> p r o", p=128)[:, r, :],
                                esum_red[:, r, :],
                            )
    _split_multi_waits(nc)
    return nc


_NC_CACHE = None


def _get_nc():
    global _NC_CACHE
    if _NC_CACHE is None:
        _NC_CACHE = build_nc()
    return _NC_CACHE


def _l2norm(x, axis=-1, eps=1e-12):
    n = np.sqrt(np.sum(x * x, axis=axis, keepdims=True))
    return x / np.maximum(n, eps)


def kernel(q, k, q_backbone, queue, queue_backbone, _trace=False, _tmpdir=None):
    qn = _l2norm(q.astype(np.float32)).reshape(N, DIM)
    kn = _l2norm(k.astype(np.float32))
    qb = _l2norm(q_backbone.astype(np.float32)).reshape(N, DIM_MLP)

    qnt_host = np.ascontiguousarray((qn / T).T)  # [128, 1280] f32, 1/T folded
    bnp = ml_dtypes.float8_e4m3 if USE_FP8 else ml_dtypes.bfloat16
    bscale = FP8_SCALE if USE_FP8 else 1.0
    qbt_host = np.ascontiguousarray((qb * bscale).T).astype(bnp)  # [2048, 1280]
    wb_bf = (queue_backbone * np.float32(bscale)).astype(bnp)

    in_maps = []
    for c in range(N_CORES):
        sl = slice(c * KSL, (c + 1) * KSL)
        in_maps.append(
            {
                "qnt": qnt_host,
                "qbt": qbt_host,
                "wq": np.ascontiguousarray(queue[:, sl]),
                "wb": np.ascontiguousarray(wb_bf[:, sl]),
            }
        )

    nc = _get_nc()
    res = run_bass_kernel_spmd(
        nc, in_maps, list(range(N_CORES)), trace=_trace, tmpdir=_tmpdir
    )

    # ---- host-side assembly ----
    l_pos = np.sum(qn.reshape(B, NCROP, DIM) * kn[:, None, :], axis=-1).reshape(N)
    logits = np.empty((N, KQ + 1), dtype=np.float32)
    logits[:, 0] = l_pos / T
    esum_tot = np.zeros((N,), dtype=np.float64)
    all_vals = []
    all_cols = []
    chunk_base = (np.arange(NCHUNK * 8, dtype=np.int64) // 8) * CHUNK
    for c in range(N_CORES):
        r = res.results[c]
        logits[:, 1 + c * KSL : 1 + (c + 1) * KSL] = r["logits_s"]
        esum_tot += r["esum"][:, 0].astype(np.float64)
        all_vals.append(r["cand_v"])
        all_cols.append(c * KSL + chunk_base[None, :] + r["cand_i"].astype(np.int64))
    vals = np.concatenate(all_vals, axis=1)  # [N, 1024]
    cols = np.concatenate(all_cols, axis=1)
    if USE_FP8:
        # fp8 values only rank candidates; rescore the top-RESCORE exactly.
        topm = np.argpartition(-vals, RESCORE, axis=1)[:, :RESCORE]
        cand_cols = np.take_along_axis(cols, topm, axis=1)  # [N, 16]
        gathered = queue_backbone[:, cand_cols.ravel()]  # [2048, N*16]
        exact = np.einsum(
            "nd,dnm->nm",
            qb,
            gathered.reshape(DIM_MLP, N, RESCORE),
            optimize=True,
        )
        top5 = np.argpartition(-exact, TOPK, axis=1)[:, :TOPK]
        sel_cols = np.take_along_axis(cand_cols, top5, axis=1)
    else:
        top5 = np.argpartition(-vals, TOPK, axis=1)[:, :TOPK]
        sel_cols = np.take_along_axis(cols, top5, axis=1)  # [N, 5] global cols
    lse = np.log(esum_tot)  # [N] float64
    pos_logp = (
        logits[np.arange(N)[:, None], 1 + sel_cols].astype(np.float64) - lse[:, None]
    )
    loss = -np.mean(np.mean(pos_logp, axis=1))

    labels = np.zeros((N,), dtype=np.int32)
    out = (logits, labels, np.float32(loss))
    if _trace:
        return out, res
    return out
